# revision 18
# baseline (speedup 1.0000x reference)
"""Trainium2 Bass kernel for nn_DLinear_436.

Mathematical reformulation
--------------------------
The reference computes, per batch b:
  1. series decomposition along time: trend = A @ x_time (A is a fixed 12x12
     matrix because the moving average with replicate padding is linear),
     res = (I - A) @ x_time
  2. unfold into 8x8 patches (stride 4), a locally-connected contraction over
     (t_in, c_in) producing O=48 maps per patch position, and fold (overlap-add)
     back to HxW.

Because the locally-connected weight for patch position (i,j) multiplies the
*same* input pixel that the fold writes back to, unfold+linear+fold collapse to
a per-pixel linear map:

  out[b, o, y, x] = sum_k x[b, k, y, x] * W_eff[o, k, y, x] + B_eff[o, y, x]

with k = (t_in, c_in) in [0,48), and W_eff depending on (y, x) only through
144 = 12x12 classes: y in {rows 0..3} u {interior phase y%4} u {rows 60..63},
same for x.  The time-mix (I-A)/A folds into the weights, so the whole module
becomes ONE 49x48 fp16 matmul per pixel (49th row = constant-1 feature carrying
the folded bias; fp16 rounding of x and W gives ~4e-4 scale-relative error
while streaming the PE at 1 cycle/column -- fp32 matmul is 4x slower).

Device mapping
--------------
131072 batch-pixels are sharded over 8 cores by (class, batch); every core gets
an identical schedule: per core two parity streams of 8192 columns each
(1 big + 4 medium + 4 small classes), mapped to the even / odd 512-column PSUM
banks.  Parity streams exist so that a run of consecutive same-parity banks
shares one stationary operand (one LDWEIGHTS per class run, back-to-back
matmuls keep the PE HAM-warm).  Even-bank matmuls write psum partitions 0..47
via a [W|0] zero-padded stationary, odd banks 48..95 via [0|W] (engine
partition windows must be 32-aligned and lane-preserving, so row packing
happens on the PE).  Only the first matmul of each [96,512] psum tile uses
start=True; later ones use start=False which overwrites where the per-element
has_written bit is clear and accumulates where set.

xin is [128, 8192] fp16: rows 0..48 = features of column-half A (banks 0..15),
rows 64..112 = half B (banks 16..31); rows 49..63 / 113..127 are zero so both
rhs windows start 32-aligned.  Each [96, 512] psum pair-tile is evacuated with
one copy (DVE / ScalarE alternating per out tile) into [96, 2048] fp32 staging
tiles, DMA'd out, and the host inverse-permutes pixels.
"""

import numpy as np
import ml_dtypes

import concourse.bass as bass
import concourse.tile as tile
from concourse import bacc, mybir
from concourse.bass_utils import run_bass_kernel_spmd

F16 = np.float16

# Problem constants (hardcoded per contract -- kernel.py must be self-contained)
B, T, H, W, C = 32, 12, 64, 64, 4
O = 48          # T_out * C_out
KF = 48         # contraction features = T * C
DEC_K = 25
N_CORES = 8
COLS = 16384    # columns (batch-pixels) per core
BANK = 512      # psum bank columns
N_BANKS = COLS // BANK          # 32 global banks; even banks = stream L rows
N_PAIRS = N_BANKS // 2          # 16 (even, odd) bank pairs
STREAM = COLS // 2              # 8192 columns per parity stream
XCOLS = 8192                    # xin columns (two 64-row halves stacked)
XCHUNK = 2048                   # xin DMA chunk columns
N_XCHUNKS = XCOLS // XCHUNK     # 4
SB_PAIRS = 4                    # pairs per superblock (psum double-buffer: 2x4)
N_SB = N_PAIRS // SB_PAIRS      # 4 superblocks
OTILE = SB_PAIRS * BANK         # 2048 out staging cols per superblock

# Per-stream class column sizes: 1 big (14*14*32), 4 medium (14*32), 4 small (32)
STREAM_SIZES = [6272] + [448] * 4 + [32] * 4
assert sum(STREAM_SIZES) == STREAM

# stationary weight layout: [Z48 | W_0 Z48 | W_1 Z48 | ... ], 64 rows fp16.
# class slot s even-parity slice = cols [48+96s, 48+96s+96) = [W_s | Z]
#              odd-parity  slice = cols [96s, 96s+96)       = [Z | W_s]
N_SLOTS = 18                    # 2 streams x 9 classes
WCOLS = 48 + 96 * N_SLOTS


def _bank_half_col(b):
    """Global bank -> (row-half h, xin col base)."""
    return b // 16, BANK * (b % 16)


# ---------------------------------------------------------------------------
# Static schedule (identical on every core)
# ---------------------------------------------------------------------------

def _class_defs():
    """The 12 y-classes (same for x): list of (kernel-index set, coord list)."""
    defs = []
    for y in range(4):                       # top border rows
        defs.append(([y], [y]))
    for p in range(4):                       # interior phases
        defs.append(([p, p + 4], [y for y in range(4, H - 4) if y % 4 == p]))
    for d in range(4):                       # bottom border rows
        defs.append(([d + 4], [H - 4 + d]))
    return defs


def _core_classes(core):
    """Two streams of 9 (cy, cx) class ids: [1 big, 4 med, 4 small] each."""
    mids = [4, 5, 6, 7]
    edges = [0, 1, 2, 3, 8, 9, 10, 11]
    bigs = [(cy, cx) for cy in mids for cx in mids]
    meds = [(cy, cx) for cy in mids for cx in edges] + \
           [(cy, cx) for cy in edges for cx in mids]
    smalls = [(cy, cx) for cy in edges for cx in edges]
    sl = [bigs[2 * core]] + meds[8 * core:8 * core + 4] \
        + smalls[8 * core:8 * core + 4]
    sr = [bigs[2 * core + 1]] + meds[8 * core + 4:8 * core + 8] \
        + smalls[8 * core + 4:8 * core + 8]
    return sl, sr


def _stream_ranges():
    """Per stream: list of (slot_within_stream, lo, hi) stream-column ranges."""
    offs = np.concatenate([[0], np.cumsum(STREAM_SIZES)])
    return [(s, int(offs[s]), int(offs[s + 1])) for s in range(len(STREAM_SIZES))]


def _pair_mms(par):
    """Per pair j: list of (slot, lo, hi) stream-column ranges intersecting
    the pair's stream window [512j, 512j+512). slot is the global weight slot
    (stream L slots 0..8, stream R slots 9..17)."""
    out = []
    for j in range(N_PAIRS):
        w0, w1 = BANK * j, BANK * (j + 1)
        mms = []
        for s, lo, hi in _stream_ranges():
            a, b = max(lo, w0), min(hi, w1)
            if a < b:
                mms.append((s + 9 * par, int(a), int(b)))
        out.append(mms)
    return out


# ---------------------------------------------------------------------------
# Device program
# ---------------------------------------------------------------------------

def _dedupe_ldweights(nc):
    """Drop InstLdweights that reload the exact stationary already in the PE
    array (Tile's lowering emits one per matmul).  Only waitless/updateless
    loads whose weights AP matches the previous load are removed; a Matmult
    with self-load or transpose clobbers the tracked state."""
    removed = 0
    for f in nc.m.functions:
        for b in f.blocks:
            last = None
            drop = []
            for i in b.instructions:
                tn = type(i).__name__
                if tn == "InstLdweights":
                    key = str(i.ins[0]), str(i.perf_mode), str(i.is_transpose)
                    if (key == last and not i.has_wait() and not i.has_update()):
                        drop.append(i)
                    else:
                        last = key
                elif tn == "InstMatmult":
                    if getattr(i, "ldweights", False) or i.is_transpose:
                        last = None
            for i in drop:
                b.instructions.remove(i)
            removed += len(drop)
    return removed


_PROG = None


def _build_program():
    global _PROG
    if _PROG is not None:
        return _PROG
    # Bacc (not raw Bass): its compile() splits multi-sem waits and moves
    # matmul waits onto ldweights -- MM/DMA HW structs allow only one wait.
    nc = bacc.Bacc("TRN2", target_bir_lowering=False, debug=False,
                   num_devices=N_CORES)
    xin_d = nc.dram_tensor("xin", [128, XCOLS], mybir.dt.float16,
                           kind="ExternalInput").ap()
    w_d = nc.dram_tensor("win", [128, WCOLS], mybir.dt.float16,
                         kind="ExternalInput").ap()
    out_d = nc.dram_tensor("out", [96, COLS // 2], mybir.dt.float32,
                           kind="ExternalOutput").ap()

    mmsL = _pair_mms(0)
    mmsR = _pair_mms(1)

    with tile.TileContext(nc) as tc:
        with tc.tile_pool(name="sb", bufs=1) as sb, \
             tc.tile_pool(name="ps", bufs=8, space="PSUM") as ps:
            wt = sb.tile([128, WCOLS], mybir.dt.float16, name="wt", tag="wt")
            nc.sync.dma_start(wt[:], w_d[:])

            xts = []
            for k in range(N_XCHUNKS):
                xt = sb.tile([128, XCHUNK], mybir.dt.float16,
                             name=f"xt{k}", tag=f"x{k}")
                nc.sync.dma_start(xt[:], xin_d[:, XCHUNK * k:XCHUNK * (k + 1)])
                xts.append(xt)

            outs = [sb.tile([96, OTILE], mybir.dt.float32,
                            name=f"ot{i}", tag=f"o{i}")
                    for i in range(N_SB)]

            # superblock order 0,2,1,3: the first two only need xin chunks 0,1
            for si, s in enumerate([0, 2, 1, 3]):
                pairs = list(range(SB_PAIRS * s, SB_PAIRS * (s + 1)))
                pts = [ps.tile([96, BANK], mybir.dt.float32,
                               name=f"pt{s}_{i}", tag="pt")
                       for i in range(SB_PAIRS)]
                started = [False] * SB_PAIRS
                n_mms = [len(mmsL[j]) + len(mmsR[j]) for j in pairs]
                done = [0] * SB_PAIRS
                # stream L (even banks, psum rows 0..47) then stream R:
                # same-slot MMs run back-to-back across pairs sharing one
                # stationary (Tile emits one LDWEIGHTS per stationary change)
                for par, mms in ((0, mmsL), (1, mmsR)):
                    # order: by slot first, then pair
                    work = []
                    for i, j in enumerate(pairs):
                        for (slot, lo, hi) in mms[j]:
                            work.append((slot, i, j, lo, hi))
                    work.sort(key=lambda t: (t[0], t[1]))
                    for (slot, i, j, lo, hi) in work:
                        bank = 2 * j + par
                        h, xbase = _bank_half_col(bank)
                        rl = xbase + lo - BANK * j
                        rh = xbase + hi - BANK * j
                        k = rl // XCHUNK
                        pl, ph = lo - BANK * j, hi - BANK * j
                        c0 = 48 * (1 - par) + 96 * slot
                        done[i] += 1
                        # stationary must share the rhs partition base:
                        # weights are duplicated in rows 64..127 for half 1
                        nc.tensor.matmul(
                            pts[i][:, pl:ph],
                            wt[64 * h:64 * h + 64, c0:c0 + 96],
                            xts[k][64 * h:64 * h + 64,
                                   rl - XCHUNK * k:rh - XCHUNK * k],
                            start=not started[i],
                            stop=done[i] == n_mms[i])
                        started[i] = True
                ot = outs[s]
                for i in range(SB_PAIRS):
                    dst = ot[:, BANK * i:BANK * (i + 1)]
                    # one engine per out tile -> single sync wait on the DMA
                    if si % 2 == 0:
                        nc.vector.tensor_copy(dst, pts[i][:, :])
                    else:
                        nc.scalar.activation(dst, pts[i][:, :],
                                             mybir.ActivationFunctionType.Copy)
                nc.scalar.dma_start(out_d[:, OTILE * s:OTILE * (s + 1)], ot[:])
    _dedupe_ldweights(nc)
    nc.compile()
    _PROG = nc
    return nc


# ---------------------------------------------------------------------------
# Host-side marshalling
# ---------------------------------------------------------------------------

def _time_mix_matrix():
    """trend[t] = sum_s A[t,s] x[s] for the DEC_K=25 replicate-pad moving avg."""
    A = np.zeros((T, T), np.float64)
    A[:, 1:11] = 1.0 / DEC_K
    for t in range(T):
        A[t, 0] = (13 - t) / DEC_K
        A[t, 11] = (t + 2) / DEC_K
    return A


def _stream_colmap(classes):
    """Per stream-column (b, y, x) arrays, in stream order (b-major per class)."""
    defs = _class_defs()
    bs, ys, xs = [], [], []
    for (cy, cx) in classes:
        yv = np.asarray(defs[cy][1])
        xv = np.asarray(defs[cx][1])
        ny, nx = len(yv), len(xv)
        bs.append(np.repeat(np.arange(B), ny * nx))
        ys.append(np.tile(np.repeat(yv, nx), B))
        xs.append(np.tile(np.tile(xv, ny), B))
    return np.concatenate(bs), np.concatenate(ys), np.concatenate(xs)


def kernel(x, w_s, b_s, w_t, b_t):
    x = np.ascontiguousarray(x, np.float32)
    A = _time_mix_matrix()
    IA = np.eye(T) - A
    # fold time mix into weights: what multiplies x[s] (source time index)
    w_hat = (np.einsum('ts,otcij->oscij', IA, w_s.astype(np.float64))
             + np.einsum('ts,otcij->oscij', A, w_t.astype(np.float64)))
    bsum = (b_s + b_t).astype(np.float64)

    # per-class effective weights: sum w_hat over the class's (i, j) rectangle
    defs = _class_defs()
    SY = np.zeros((12, 8), np.float64)
    for ci, (iset, _) in enumerate(defs):
        SY[ci, iset] = 1.0
    Wall = np.einsum('yi,xj,oscij->yxosc', SY, SY, w_hat)   # [12,12,48,12,4]
    Ball = np.einsum('yi,xj,oij->yxo', SY, SY, bsum)        # [12,12,48]

    x_r = x.transpose(1, 4, 0, 2, 3).reshape(KF, B, H, W)   # [k=(t,c), b, y, x]

    # stream column -> xin position: pair j = col//512, bank = 2j+par,
    # half h = bank//16, xin col = 512*(bank%16) + col%512
    j = np.arange(STREAM) // BANK
    off = np.arange(STREAM) % BANK
    xin_pos = []
    for par in range(2):
        bank = 2 * j + par
        xin_pos.append((bank // 16, BANK * (bank % 16) + off))

    in_maps = []
    colmaps = []
    for core in range(N_CORES):
        sl, sr = _core_classes(core)
        xin = np.zeros((128, XCOLS), F16)
        cms = []
        for par, classes in ((0, sl), (1, sr)):
            barr, yarr, xarr = _stream_colmap(classes)
            cms.append((barr, yarr, xarr))
            feats = np.empty((49, STREAM), np.float32)
            feats[:KF] = x_r[:, barr, yarr, xarr]
            feats[KF] = 1.0
            h, xc = xin_pos[par]
            xin[(64 * h)[None, :] + np.arange(49)[:, None], xc[None, :]] = \
                feats.astype(F16)
        colmaps.append(cms)

        wm = np.zeros((128, WCOLS), F16)
        for slot, (cy, cx) in enumerate(sl + sr):
            M = np.empty((49, 48), np.float64)
            M[:KF] = Wall[cy, cx].reshape(O, KF).T           # rows k=(t,c), cols o
            M[KF] = Ball[cy, cx]
            c0 = 48 + 96 * slot
            wm[0:49, c0:c0 + 48] = M.astype(F16)
        wm[64:128] = wm[0:64]
        in_maps.append({"xin": xin, "win": wm})

    nc = _build_program()
    res = run_bass_kernel_spmd(nc, in_maps, core_ids=list(range(N_CORES)))
    per_core = res.results if hasattr(res, "results") else res

    # gather: stream col -> out_dram[48*par + o, 512*(col//512) + col%512]
    scol = np.arange(STREAM)
    dcol = BANK * (scol // BANK) + scol % BANK  # = scol (pairs map 1:1 to cols)
    rows_o = np.arange(O)[:, None]
    out_boyx = np.empty((B, O, H, W), np.float32)
    for core in range(N_CORES):
        od = per_core[core]["out"]
        for par in range(2):
            vals = od[48 * par + rows_o, dcol[None, :]]      # [48, STREAM]
            barr, yarr, xarr = colmaps[core][par]
            out_boyx[barr, :, yarr, xarr] = vals.T
    out = out_boyx.reshape(B, 12, 4, H, W).transpose(0, 1, 3, 4, 2)
    return np.ascontiguousarray(out)
